# revision 36
# baseline (speedup 1.0000x reference)
"""Multi-head attention (B=2, S=2048, E=2048, H=16, causal) on 8 TRN2 NeuronCores.

Sharding: 8 cores = 2 batch shards x 4 head-group shards (4 heads / 512
features each).  Each core runs the full attention stack for its (batch,
head-group) and produces a partial [S, E] output through its row-block of
Wo; the host sums the 4 partials per batch.

All matmuls run as float32r (full PE rate for free dim >= 256).
"""

import numpy as np

import concourse.bacc as bacc
import concourse.mybir as mybir
import concourse.tile as tile
from concourse import bass_utils

B, S, E, H = 2, 2048, 2048, 16
D = 128                    # head dim
HL = 4                     # heads per core
F = HL * D                 # local features = 512
EO = E // 128              # 16 contraction chunks
EG = 2                     # eo chunks per DMA group
TT = 256                   # phase-1 token tile
IT = 512                   # phase-2 query tile
F32 = mybir.dt.float32
F32R = mybir.dt.float32r
EXP = mybir.ActivationFunctionType.Exp
SCALE = 1.0 / float(np.sqrt(D))

_CACHE = {}


def _build():
    nc = bacc.Bacc("TRN2", target_bir_lowering=False, debug=False)
    xT = nc.dram_tensor("xT", [E, S], F32, kind="ExternalInput").ap()
    wqT = nc.dram_tensor("wqT", [E, F], F32, kind="ExternalInput").ap()
    wkT = nc.dram_tensor("wkT", [E, F], F32, kind="ExternalInput").ap()
    wvT = nc.dram_tensor("wvT", [E, F], F32, kind="ExternalInput").ap()
    woT = nc.dram_tensor("woT", [F, E], F32, kind="ExternalInput").ap()
    # causal mask pairs: [pair, 128, 2, IT]
    cmask = nc.dram_tensor("cmask", [2, 128, 2, IT], F32, kind="ExternalInput").ap()
    y = nc.dram_tensor("y", [S, E], F32, kind="ExternalOutput").ap()

    xT_t = xT.rearrange("(eo ei) t -> ei eo t", ei=128).bitcast(F32R)
    wqT_t = wqT.rearrange("(eo ei) f -> ei eo f", ei=128).bitcast(F32R)
    wkT_t = wkT.rearrange("(eo ei) f -> ei eo f", ei=128).bitcast(F32R)
    wvT_t = wvT.rearrange("(eo ei) f -> ei eo f", ei=128).bitcast(F32R)
    woT_t = woT.rearrange("(fc fi) e -> fi fc e", fi=128).bitcast(F32R)

    with tile.TileContext(nc) as tc:
        with tc.tile_pool(name="persist", bufs=1) as persist:
            qT = persist.tile([128, HL, S], F32R, tag="qT")
            kT = persist.tile([128, HL, S], F32R, tag="kT")
            vN = persist.tile([128, S // 128, F], F32R, tag="vN")
            maskT = persist.tile([128, 2, 2, IT], F32, tag="maskT")
            onesT_f = persist.tile([128, 1], F32, tag="onesT_f")
            onesT = persist.tile([128, 1], F32R, tag="onesT")

            nc.vector.memset(onesT_f[:], 1.0)
            nc.vector.tensor_copy(onesT[:], onesT_f[:])

            # ---------- phase 1: q/k/v projections (two f-half passes) ----
            with (
                tc.tile_pool(name="wres_q", bufs=2) as wpool_q,
                tc.tile_pool(name="wres_kv", bufs=1) as wpool_kv,
                tc.tile_pool(name="xstream", bufs=2) as xpool,
                tc.tile_pool(name="ps_qk", bufs=5, space="PSUM") as ps_qk,
                tc.tile_pool(name="ps_v", bufs=3, space="PSUM") as ps_v,
            ):
                for fp in range(2):
                    f0 = fp * 256
                    wq_res = wpool_q.tile([128, EO, 256], F32R, tag="wq")
                    wk_res = wpool_kv.tile([128, EO, 256], F32R, tag="wk")
                    wv_res = wpool_kv.tile([128, EO, 256], F32R, tag="wv")
                    xt0 = xpool.tile([128, EO, TT], F32R, tag="xt")
                    # issue in consumption order: wq/x first, then wk, wv
                    for g0 in range(0, EO, EG):
                        nc.sync.dma_start(
                            wq_res[:, g0:g0 + EG, :],
                            wqT_t[:, g0:g0 + EG, f0:f0 + 256],
                        )
                        nc.sync.dma_start(
                            xt0[:, g0:g0 + EG, :], xT_t[:, g0:g0 + EG, 0:TT]
                        )
                    for g0 in range(0, EO, EG):
                        nc.sync.dma_start(
                            wk_res[:, g0:g0 + EG, :],
                            wkT_t[:, g0:g0 + EG, f0:f0 + 256],
                        )
                    for g0 in range(0, EO, EG):
                        nc.sync.dma_start(
                            wv_res[:, g0:g0 + EG, :],
                            wvT_t[:, g0:g0 + EG, f0:f0 + 256],
                        )
                    for tt in range(S // TT):
                        t0 = tt * TT
                        if fp == 0 and tt == 4:
                            nc.sync.dma_start(
                                maskT[:], cmask.rearrange("q p m i -> p q m i")
                            )
                        if tt == 0:
                            xt = xt0
                        else:
                            xt = xpool.tile([128, EO, TT], F32R, tag="xt")
                            for g0 in range(0, EO, EG):
                                nc.sync.dma_start(
                                    xt[:, g0:g0 + EG, :],
                                    xT_t[:, g0:g0 + EG, t0:t0 + TT],
                                )
                        for wres, dst in ((wq_res, qT), (wk_res, kT)):
                            for fc in range(2):
                                ps = ps_qk.tile([128, TT], F32, tag="pqk")
                                for eo in range(EO):
                                    nc.tensor.matmul(
                                        ps[:],
                                        wres[:, eo, fc * 128:(fc + 1) * 128],
                                        xt[:, eo, :],
                                        start=(eo == 0),
                                        stop=(eo == EO - 1),
                                    )
                                nc.vector.tensor_copy(
                                    dst[:, fp * 2 + fc, t0:t0 + TT], ps[:]
                                )
                        for tc2 in range(TT // 128):
                            ps = ps_v.tile([128, 256], F32, tag="pv")
                            for eo in range(EO):
                                nc.tensor.matmul(
                                    ps[:],
                                    xt[:, eo, tc2 * 128:(tc2 + 1) * 128],
                                    wv_res[:, eo, :],
                                    start=(eo == 0),
                                    stop=(eo == EO - 1),
                                )
                            nc.vector.tensor_copy(
                                vN[:, (t0 // 128) + tc2, f0:f0 + 256], ps[:]
                            )

            # ---------- phase 2: attention per head ----------------------
            with tc.tile_pool(name="wo", bufs=1) as wo_pool:
                wo_res = wo_pool.tile([128, HL, E], F32R, tag="wo")
                outT = wo_pool.tile([128, HL, S], F32R, tag="outT")
                for g0 in range(0, HL, 2):
                    nc.sync.dma_start(
                        wo_res[:, g0:g0 + 2, :], woT_t[:, g0:g0 + 2, :]
                    )

                with (
                    tc.tile_pool(name="ph2", bufs=6) as epool,
                    tc.tile_pool(name="ph2t", bufs=4) as tpool,
                    tc.tile_pool(name="ph2b", bufs=2) as small,
                    tc.tile_pool(name="ps_s", bufs=5, space="PSUM") as ps_s,
                    tc.tile_pool(name="ps_o", bufs=2, space="PSUM") as ps_o,
                    tc.tile_pool(name="ps_r", bufs=1, space="PSUM") as ps_r,
                ):
                    for h in range(HL):
                        h0 = h * 128
                        for p in range(S // IT):
                            i0 = p * IT
                            njc = (i0 + IT) // 128
                            O = ps_o.tile([128, IT], F32, tag="O")
                            R = ps_r.tile([1, IT], F32, tag="R")

                            def emit_scores(jc):
                                q_off = jc - (i0 // 128)
                                # diag chunk q: columns i < 128*q are fully
                                # masked -- compute only the valid slice
                                # (clamped so the free dim stays >= 256 for
                                # the f32r fast path)
                                off = 0 if q_off < 0 else min(128 * q_off, 256)
                                Sps = ps_s.tile([128, IT], F32, tag="S")
                                nc.tensor.matmul(
                                    Sps[:, off:],
                                    kT[:, h, jc * 128:(jc + 1) * 128],
                                    qT[:, h, i0 + off:i0 + IT],
                                    start=True,
                                    stop=True,
                                )
                                Et = epool.tile([128, IT], F32R, tag="E")
                                if q_off < 0:
                                    nc.scalar.activation(
                                        Et[:], Sps[:], EXP, scale=SCALE
                                    )
                                else:
                                    Etmp = tpool.tile([128, IT], F32, tag="Etmp")
                                    nc.scalar.activation(
                                        Etmp[:, off:], Sps[:, off:], EXP,
                                        scale=SCALE,
                                    )
                                    nc.vector.tensor_mul(
                                        Et[:, off:], Etmp[:, off:],
                                        maskT[:, q_off // 2, q_off % 2, off:],
                                    )
                                return Et, off

                            def emit_av(jc, Et, off):
                                nc.tensor.matmul(
                                    O[:, off:],
                                    vN[:, jc, h0:h0 + 128],
                                    Et[:, off:],
                                    start=(jc == 0),
                                    stop=(jc == njc - 1),
                                )
                                nc.tensor.matmul(
                                    R[:, off:],
                                    onesT[:],
                                    Et[:, off:],
                                    start=(jc == 0),
                                    stop=(jc == njc - 1),
                                )

                            # scores/exp run 4 chunks ahead of attn@v/rowsum
                            pending = []
                            for jc in range(njc):
                                Et, off = emit_scores(jc)
                                pending.append((jc, Et, off))
                                if len(pending) > 4:
                                    emit_av(*pending.pop(0))
                            for item in pending:
                                emit_av(*item)
                            rec = small.tile([1, IT], F32, tag="rec")
                            nc.vector.reciprocal(rec[:], R[:])
                            RB = small.tile([128, IT], F32, tag="RB")
                            nc.gpsimd.partition_broadcast(RB[:], rec[:])
                            if h == HL - 1 and p == S // IT - 1:
                                # free the last O/R banks early so phase-3's
                                # psum pool isn't gated on the recip chain
                                Ocp = small.tile([128, IT], F32, tag="Ocp")
                                nc.vector.tensor_copy(Ocp[:], O[:])
                                nc.vector.tensor_mul(
                                    outT[:, h, i0:i0 + IT], Ocp[:], RB[:]
                                )
                            else:
                                nc.vector.tensor_mul(
                                    outT[:, h, i0:i0 + IT], O[:], RB[:]
                                )

                # ------ phase 3: output projection ------------------------
                with (
                    tc.tile_pool(name="yst3", bufs=6) as yst_pool,
                    tc.tile_pool(name="ps_y", bufs=8, space="PSUM") as ps_y,
                ):
                    for tcb in range(S // 128):
                        tb0 = tcb * 128
                        for et in range(E // 512):
                            Y = ps_y.tile([128, 512], F32, tag="Y")
                            for fc in range(HL):
                                nc.tensor.matmul(
                                    Y[:],
                                    outT[:, fc, tb0:tb0 + 128],
                                    wo_res[:, fc, et * 512:(et + 1) * 512],
                                    start=(fc == 0),
                                    stop=(fc == HL - 1),
                                )
                            yst = yst_pool.tile([128, 512], F32, tag="yst")
                            if et % 2 == 1:
                                nc.vector.tensor_copy(yst[:], Y[:])
                            else:
                                nc.scalar.copy(yst[:], Y[:])
                            nc.sync.dma_start(
                                y[tb0:tb0 + 128,
                                  et * 512:(et + 1) * 512],
                                yst[:],
                            )
    nc.compile()
    return nc


def _get_nc():
    if "nc" not in _CACHE:
        _CACHE["nc"] = _build()
    return _CACHE["nc"]


def make_in_maps(x, Wq, Wk, Wv, Wo):
    x = np.asarray(x, np.float32)
    Wq = np.asarray(Wq, np.float32)
    Wk = np.asarray(Wk, np.float32)
    Wv = np.asarray(Wv, np.float32)
    Wo = np.asarray(Wo, np.float32)

    jj = np.arange(128, dtype=np.int64)[:, None]
    ii = np.arange(IT, dtype=np.int64)[None, :]
    cm = np.stack(
        [(128 * q + jj <= ii).astype(np.float32) for q in range(4)]
    )  # [4, 128, IT]
    cmask = np.ascontiguousarray(
        cm.reshape(2, 2, 128, IT).transpose(0, 2, 1, 3)
    )  # [pair, 128, 2, IT]

    xTs = [np.ascontiguousarray(x[b].T) for b in range(B)]
    in_maps = []
    for c in range(8):
        b, g = c // 4, c % 4
        fsl = slice(F * g, F * (g + 1))
        in_maps.append({
            "xT": xTs[b],
            "wqT": np.ascontiguousarray(Wq[fsl, :].T),
            "wkT": np.ascontiguousarray(Wk[fsl, :].T),
            "wvT": np.ascontiguousarray(Wv[fsl, :].T),
            "woT": np.ascontiguousarray(Wo[:, fsl].T),
            "cmask": cmask,
        })
    return in_maps


def combine_outputs(results):
    out = np.empty((B, S, E), np.float32)
    for b in range(B):
        acc = results[4 * b]["y"].astype(np.float32).copy()
        for g in range(1, 4):
            acc += results[4 * b + g]["y"]
        out[b] = acc
    return out


def kernel(x, Wq, Wk, Wv, Wo):
    nc = _get_nc()
    in_maps = make_in_maps(x, Wq, Wk, Wv, Wo)
    res = bass_utils.run_bass_kernel_spmd(nc, in_maps, core_ids=list(range(8)))
    return combine_outputs(res.results)
